# revision 33
# baseline (speedup 1.0000x reference)
"""OccupancyToTopology Trainium2 kernel (bf16 2x pipeline, v6).

Input: occupancy [65, 65, 65] f32 on a (W+1,H+1,D+1) grid, W=H=D=64.
Output: topo [262144, 256] f32 where topo[n, t] = prod_c (p_c if bit_c(t) else 1-p_c),
with n = x*4096 + y*64 + z and the 8 cell corners in marching-cubes order
  CORNER_OFFSETS = [(0,0,0),(1,0,0),(1,1,0),(0,1,0),(0,0,1),(1,0,1),(1,1,1),(0,1,1)]
(offsets are (dx,dy,dz); bit c of t selects corner c).

Sharding: x split across 8 cores; core k owns cells x in [8k, 8k+8) and gets the
occupancy slab occupancy[8k:8k+9] (1-plane halo). Output rows are fully local.

Per-core pipeline (partitions p = x2*64 + y for a group of two x-planes):
  gathers:  TWO DMAs per group (one per x2 half): rab layout (dx2, dy2, z65);
            dy and z merge into one contiguous 130-f32 run of occ, giving the
            3-dim AP [y64, dx2, dyz130] (DMA APs cap at 3 dims).
  terms (ScalarE, 8 ACTs split by (dy, oz, b)): T8 (half2, row2, oz2, z64, b2)
            f32, b=0 half 1-p, b=1 half p. The (dx,dy) -> (half,row) corner
            mapping is affine only per fixed dy (dx-stride +-512), hence the
            split. Group 0's b=1 half goes to DVE copies (idle in the head).
  pairs (1 TT f32, DVE):   P4ALL (s4, z64, bh2, bl2), slot s=(row,oz) =
            [pair01, pair45, pair23, pair67] (bit pairs of t)
  quads (2 TT f32->bf16):  Q16ALL (lh2, z64, jh4, jl4); lh=0 = L16 (bits 0-3)
            = slots 0x2, lh=1 = H16 (bits 4-7) = slots 1x3. 1x mode: one quad
            operand always has a broadcast innermost dim, and z-innermost
            layouts are barred by the store (t must be HBM-contiguous).
  dup (ScalarE):           H16 -> H16D (z64, h16, d2) so the combine's h
            operand has a packed innermost pair (2x-mode requirement)
  combine (TT bf16 @2x, DVE): per z16 chunk OUT[z, h, l] = L16[z,l]*H16D[z,h]

Schedule (trace-driven):
  - DVE is the bottleneck engine (~50us busy, saturated end to end): 16 z16
    combine chunks (35us) + pairs/quads (14us). Group g+1's staging ops are
    emitted between group g's combine chunks.
  - Stores: ONE queue per z16 chunk, rotating Sync -> GpSimd -> Scalar.
    Per-queue throughput is latency-bound (~1 outstanding DMA, ~1.5us
    trigger+sem gap per piece), so whole-z16 pieces (4.2us of transfer)
    amortize the gap and ~2.7 queues stay in flight concurrently. The final
    group tapers to two z8 chunks so the exposed terminal drain is small.
  - Group 0's quads/dup are z16-sliced so the first combine only waits on
    the z0:16 slices (~2.6us earlier start); the z16:64 remainders follow
    right after it and the whole saturated DVE pipeline shifts left.
  - All engines idle through the fixed ~7.3us NEFF preamble; first gather
    data lands ~10.6us, first combine ~14.5us, last combine ~62us, stores
    drain ~5-8us past it.

Error: two quad outputs + combine output rounded to bf16 -> 3 truncation
units ~ 1.07e-2 max rel err measured, inside the 2e-2 gate (an all-bf16
tree accumulates 15 units ~ 3.4e-2: measured, fails).

Measured: ~71.3-72.4us on 8 cores (baseline 80.8us; DVE p-state luck adds
up to +8us on unlucky runs).
"""

import sys

if "/opt/trn_rl_repo" not in sys.path:
    sys.path.insert(0, "/opt/trn_rl_repo")

import numpy as np

import concourse.bass as bass
import concourse.mybir as mybir
from concourse.bass_utils import run_bass_kernel_spmd
from concourse.tile import TileContext

F32 = mybir.dt.float32
BF16 = mybir.dt.bfloat16
N_CORES = 8
W = H = D = 64
XPC = W // N_CORES          # x-planes of cells per core = 8
N_LOCAL = XPC * H * D       # cells per core = 32768
N_GROUPS = XPC // 2         # two x-planes of cells per group = 4
XS = (H + 1) * (D + 1)      # occupancy x-plane stride (elements)


def _hoist_extra_waits(nc):
    """Walrus on this toolchain rejects instructions carrying more than one
    sync-wait. Hoist every wait of a multi-wait instruction into standalone
    EventSemaphore instructions just before it in the same engine stream."""
    ctr = 0
    for fn in nc.m.functions:
        for blk in fn.blocks:
            new_insts = []
            for inst in blk.instructions:
                si = inst.sync_info
                waits = list(si.on_wait) if (si is not None and si.on_wait) else []
                if len(waits) > 1:
                    # DMA-vs-DMA ordering guards (DMAHW/DMASW lane sems) stay
                    # on the DMA itself; everything else becomes a standalone
                    # sequencer wait right before it.
                    keep = []
                    if inst.opcode in ("DMACopy", "TensorLoad", "TensorSave"):
                        for w in waits:
                            if "DMAHW" in w.ant_name or "DMASW" in w.ant_name:
                                keep = [w]
                                break
                    if not keep:
                        keep = [waits[-1]]
                    hoisted = [w for w in waits if w is not keep[0]]
                    for w in hoisted:
                        ev = mybir.InstEventSemaphore(
                            name=f"hoistw-{ctr}", ins=[], outs=[])
                        ctr += 1
                        ev.engine = inst.engine
                        ev.sync_info = mybir.SyncInfo(on_wait=[w], on_update=[])
                        new_insts.append(ev)
                    inst.sync_info = mybir.SyncInfo(
                        on_wait=keep, on_update=list(si.on_update))
                new_insts.append(inst)
            blk.instructions = new_insts


def _build_program(hoist=True):
    nc = bass.Bass()
    occ = nc.dram_tensor("occ", [XPC + 1, H + 1, D + 1], F32, kind="ExternalInput")
    topo = nc.dram_tensor("topo", [N_LOCAL, 256], BF16, kind="ExternalOutput")
    topo_ap = topo[:, :]
    occ_ap = occ[:, :, :]
    Copy = mybir.ActivationFunctionType.Copy

    with TileContext(nc) as tc:
        with (
            tc.tile_pool(name="raw", bufs=1) as raw_pool,
            tc.tile_pool(name="term", bufs=2) as term_pool,
            tc.tile_pool(name="stage", bufs=2) as stage_pool,
            tc.tile_pool(name="out", bufs=3) as out_pool,
        ):
            # ---- gathers: TWO DMAs per group (was 4). rab layout is
            # (dx2, dy2, z65); the dy and z dims merge into one contiguous
            # 130-f32 run in occ (row y then row y+1), so each DMA is the
            # 3-dim AP [y-partitions64, dx2, dyz130] for one x2 half.
            # Group 0 rides sync+gpsimd in parallel (first data in).
            rabs = []
            occ_t = occ_ap.tensor
            for g in range(N_GROUPS):
                x0 = g * 2
                rab = raw_pool.tile([128, 4 * 65], F32, tag=f"rab{g}")
                rab_ap = rab[:, :]
                engs = (nc.sync, nc.gpsimd) if g == 0 else (
                    (nc.gpsimd, nc.gpsimd) if g == 1 else (nc.sync, nc.sync))
                for x2 in (0, 1):
                    dst = bass.AP(
                        tensor=rab_ap.tensor,
                        offset=rab_ap.offset + x2 * 64 * 260,
                        ap=[[260, 64], [130, 2], [1, 130]],
                    )
                    srcap = bass.AP(
                        tensor=occ_t,
                        offset=occ_ap.offset + (x0 + x2) * XS,
                        ap=[[H + 1, 64], [XS, 2], [1, 130]],
                    )
                    engs[x2].dma_start(out=dst, in_=srcap)
                rabs.append(rab)

            # warm the ScalarE activation table while gathers are in flight
            # (the first ACTIVATE otherwise pays a ~1.3us ACT_TABLE_LOAD on
            # the critical path).
            warm = raw_pool.tile([128, 2], F32, tag="warm")
            nc.vector.memset(warm[:, 0:1], 0.0)
            nc.scalar.activation(warm[:, 1:2], warm[:, 0:1], Copy)

            def emit_terms(g):
                """terms (f32): b=0 half is 1-p, b=1 half is p. rab rows are
                (dx, dy); t8 wants (half, row) = lo{c0,c2}/hi{c1,c3} rows:
                  (dx,dy)=(0,0)->c0(half0,row0)  (1,1)->c2(half0,row1)
                  (1,0)->c1(half1,row0)          (0,1)->c3(half1,row1)
                Affine per fixed dy (dx-stride +512 / -512 elems), so the
                ACTs split by (dy, oz, b): 8 ops of 2x64 elems. Group 0's
                p-halves go to DVE (idle during the head)."""
                rab_v = rabs[g].rearrange("p (dx dy z) -> p dx dy z",
                                          dx=2, dy=2)
                t8 = term_pool.tile([128, 2 * 4 * 64 * 2], F32, tag="t8")
                t8_ap = t8[:, :]

                def dst_ap(dy, oz, b):
                    # (half,row,oz,z,b) strides 512,256,128,2,1; dx-stride
                    # +-512 with base the (dx=0,dy) row's slot
                    base = (768 if dy else 0) + oz * 128 + b
                    return bass.AP(
                        tensor=t8_ap.tensor,
                        offset=t8_ap.offset + base,
                        ap=[[1024, 128], [512 if dy == 0 else -512, 2],
                            [2, 64], [0, 1]],
                    )

                for oz in (0, 1):
                    for dy in (0, 1):
                        srcv = rab_v[:, :, dy:dy + 1, oz:oz + 64, None]
                        nc.scalar.activation(dst_ap(dy, oz, 0), srcv,
                                             Copy, bias=1.0, scale=-1.0)
                        if g == 0:
                            nc.vector.tensor_copy(dst_ap(dy, oz, 1), srcv)
                        else:
                            nc.scalar.activation(dst_ap(dy, oz, 1), srcv,
                                                 Copy)
                return t8

            class Stage:
                pass

            def emit_pair(g, t8):
                """pairs: ONE TT op, f32.
                P4ALL[s, z, bh, bl] = T8[lo, s, z, bl] * T8[hi, s, z, bh]
                (full z only: the (s,z) dim merge that keeps the operand APs
                within the 3-dim codegen limit fails for z slices)"""
                st = Stage()
                st.p4all = stage_pool.tile([128, 4 * 64 * 4], F32,
                                           tag="p4all")
                st.q16all = stage_pool.tile([128, 2 * 64 * 16], BF16,
                                            tag="q16all")
                st.h16d = stage_pool.tile([128, 64 * 16 * 2], BF16,
                                          tag="h16d")
                p4_v = st.p4all.rearrange("p (s z bh bl) -> p s z bh bl",
                                          s=4, z=64, bh=2, bl=2)[:, None]
                t8_s = t8.rearrange("p (rh s z b) -> p rh s z b",
                                    rh=2, s=4, z=64, b=2)
                lo_v = t8_s[:, 0:1, :, :, None, :] \
                    .broadcast_to([128, 1, 4, 64, 2, 2])
                hi_v = t8_s[:, 1:2, :, :, :, None] \
                    .broadcast_to([128, 1, 4, 64, 2, 2])
                nc.vector.tensor_mul(p4_v, lo_v, hi_v)
                return st

            def emit_quad(g, st, lh, z0=0, zn=64):
                """quads: per-lh TT, f32 in -> bf16 out, H (lh=1) first so
                its dup can run while the L quad executes.
                Q16ALL[lh, z, jh, jl] = P4ALL[lh, z, jl] * P4ALL[2+lh, z, jh]
                """
                q16_v = st.q16all.rearrange("p (lh z jh jl) -> p lh z jh jl",
                                            lh=2, z=64, jh=4, jl=4)
                p4_s = st.p4all.rearrange("p (s z j) -> p s z j",
                                          s=4, z=64, j=4)
                ql_v = p4_s[:, lh:lh + 1, z0:z0 + zn, None, :] \
                    .broadcast_to([128, 1, zn, 4, 4])
                qh_v = p4_s[:, 2 + lh:3 + lh, z0:z0 + zn, :, None] \
                    .broadcast_to([128, 1, zn, 4, 4])
                nc.vector.tensor_mul(q16_v[:, lh:lh + 1, z0:z0 + zn],
                                     ql_v, qh_v)

            def emit_dup(g, st, z0, zn=32):
                """combine-hi dup: H16 (lh=1) -> (z, h16, d2). ScalarE for
                steady groups; DVE tensor_copy for group 0 where the list
                scheduler otherwise parks the dup behind stalled
                instructions on the head chain."""
                h16d_dst = st.h16d.rearrange("p (z h d) -> p z h d",
                                             z=64, h=16, d=2)
                q16_zh = st.q16all.rearrange("p (lh z h) -> p lh z h",
                                             lh=2, z=64, h=16)
                h16_src = q16_zh[:, 1:2, z0:z0 + zn, :, None] \
                    .broadcast_to([128, 1, zn, 16, 2])
                nc.scalar.activation(h16d_dst[:, None, z0:z0 + zn],
                                     h16_src, Copy)

            def emit_comb(g, st, c0, cn, ci):
                """final combine @2x + store for one z-chunk.
                OUT[z, h, l] = L16[z, l] * H16D[z, h, .]"""
                if ci == 0:
                    st.out_t = out_pool.tile([128, D * 256], BF16, tag="topo")
                out_zv = st.out_t.rearrange("p (z h l8 l2) -> p z h l8 l2",
                                            z=D, h=16, l8=8, l2=2)
                q16_zl = st.q16all.rearrange("p (lh z l8 l2) -> p lh z l8 l2",
                                             lh=2, z=64, l8=8, l2=2)
                h16d_v = st.h16d.rearrange("p (z h d) -> p z h d",
                                           z=64, h=16, d=2)
                l_v = q16_zl[:, 0, c0:c0 + cn][:, :, None, :, :] \
                    .broadcast_to([128, cn, 16, 8, 2])
                h_v = h16d_v[:, c0:c0 + cn][:, :, :, None, :] \
                    .broadcast_to([128, cn, 16, 8, 2])
                nc.vector.tensor_mul(out_zv[:, c0:c0 + cn], l_v, h_v)
                # store rows (x2, y, s0..s0+sn): per partition sn/2 KiB
                # contiguous in HBM at (x2*4096 + y*64 + s0)*256 elements.
                qeng = (nc.sync, nc.gpsimd, nc.scalar)
                parts = [(s0, sn, qeng[q]) for s0, sn, q in
                         STORE_PLAN[(g, ci)]]
                for s0, sn, st_eng in parts:
                    dst = bass.AP(
                        tensor=topo_ap.tensor,
                        offset=topo_ap.offset + (g * 2 * H * D + s0) * 256,
                        ap=[[4096 * 256, 2], [D * 256, H], [1, sn * 256]],
                    )
                    st_eng.dma_start(
                        out=dst,
                        in_=st.out_t[:, s0 * 256:(s0 + sn) * 256],
                    )

            def chunks(g):
                # z16 chunks: a combine completes (releasing ~1MB) every
                # ~2.2us. The final group tapers to z8 chunks so the last
                # exposed store is small.
                if g == N_GROUPS - 1:
                    return [(0, 16), (16, 16), (32, 16), (48, 8), (56, 8)]
                return [(0, 16), (16, 16), (32, 16), (48, 16)]

            # Store schedule: ONE queue per z16 chunk (an unsplit z16 piece
            # amortizes the ~1.5us per-DMA trigger+sem gap: ~184 GB/s/queue
            # vs ~146 for z8 halves), rotating SY->GP->SC so ~2.7 queues are
            # in flight at any time. The final z8 chunks split z4+z4 across
            # two queues so the terminal exposure is ~2.5us.
            SY, GP, SC = 0, 1, 2
            STORE_PLAN = {
                (0, 0): ((0, 16, SY),),
                (0, 1): ((16, 16, GP),),
                (0, 2): ((32, 16, SC),),
                (0, 3): ((48, 16, SY),),
                (1, 0): ((0, 16, GP),),
                (1, 1): ((16, 16, SY),),
                (1, 2): ((32, 16, SC),),
                (1, 3): ((48, 16, GP),),
                (2, 0): ((0, 16, SY),),
                (2, 1): ((16, 16, GP),),
                (2, 2): ((32, 16, SC),),
                (2, 3): ((48, 16, SY),),
                (3, 0): ((0, 16, GP),),
                (3, 1): ((16, 16, SY),),
                (3, 2): ((32, 16, SC),),
                (3, 3): ((48, 8, SY),),
                (3, 4): ((56, 8, GP),),
            }

            # Software-pipelined emission: group g+1's stage ops (pair,
            # quads, dups, terms) are emitted BETWEEN group g's combine
            # chunks so DVE production never pauses more than one stage op.
            t8s = {0: emit_terms(0)}
            stages = {}
            # z16-sliced head quads + dup: the first combine chunk only needs
            # z0:16, so it starts ~2.6us sooner; the z16:64 remainders are
            # emitted right after it (the DVE pipeline just shifts left).
            stages[0] = emit_pair(0, t8s[0])
            emit_quad(0, stages[0], 1, 0, 16)
            emit_dup(0, stages[0], 0, 16)
            emit_quad(0, stages[0], 0, 0, 16)
            t8s[1] = emit_terms(1)
            for g in range(N_GROUPS):
                cl = chunks(g)
                last = len(cl) - 1
                for ci, (c0, cn) in enumerate(cl):
                    emit_comb(g, stages[g], c0, cn, ci)
                    if g == 0 and ci == 0:
                        emit_quad(0, stages[0], 1, 16, 48)
                        emit_dup(0, stages[0], 16, 16)
                        emit_dup(0, stages[0], 32, 32)
                        emit_quad(0, stages[0], 0, 16, 48)
                    if g + 1 < N_GROUPS:
                        if ci == 0:
                            stages[g + 1] = emit_pair(g + 1, t8s[g + 1])
                        elif ci == 1:
                            emit_quad(g + 1, stages[g + 1], 1)
                            emit_dup(g + 1, stages[g + 1], 0)
                        elif ci == 2:
                            if g + 2 < N_GROUPS:
                                t8s[g + 2] = emit_terms(g + 2)
                            emit_quad(g + 1, stages[g + 1], 0)
                        if ci == last:
                            emit_dup(g + 1, stages[g + 1], 32)

    if hoist:
        _hoist_extra_waits(nc)
    return nc


_NC_CACHE = None


def _get_program():
    global _NC_CACHE
    if _NC_CACHE is None:
        _NC_CACHE = _build_program()
    return _NC_CACHE


def kernel(occupancy: np.ndarray) -> np.ndarray:
    occupancy = np.asarray(occupancy, dtype=np.float32)
    assert occupancy.shape == (65, 65, 65)
    nc = _get_program()
    in_maps = [
        {"occ": np.ascontiguousarray(occupancy[8 * k:8 * k + 9])}
        for k in range(N_CORES)
    ]
    res = run_bass_kernel_spmd(nc, in_maps, core_ids=list(range(N_CORES)))
    return np.concatenate(
        [np.asarray(res.results[k]["topo"]).astype(np.float32)
         for k in range(N_CORES)], axis=0)

